# revision 1
# baseline (speedup 1.0000x reference)
"""GCN (3-layer GCNConv + linear) distributed Bass kernel for 8 TRN2 NeuronCores.

Algorithm notes
---------------
Reference per layer: out = D^-1/2 (A + I) D^-1/2 (x @ W) + b, relu (layers 1-3),
then final dense h @ Wf + bf. D = in-degree(+1) diag computed from dst only.

The normalization factorizes: with g = dinv * (x @ W) (row-scaled),
    out_i = dinv_i * ( sum_{e: dst=i} g[src_e]  +  g_i ) + b
so no per-edge weights are needed on device - only gather indices.

Sharding: nodes are dealt round-robin by in-degree rank to the 8 cores, so
each core owns 12500 nodes (padded to 98*128=12544) sorted by in-degree.
Per layer each core:
  A) computes g for its own nodes (small PE matmul),
  B) AllGathers g into a full table [8*12544, F] in DRAM,
  C) ELL-gathers g[src] rows per 128-node tile via indirect DMA and
     tree-accumulates on DVE, applies dinv/bias/relu.
Degree-sorting makes per-tile ELL padding tiny.
"""

import numpy as np
import sys
for _p in ("/opt/trn_rl_repo", "/root/.axon_site/_ro/trn_rl_repo"):
    if _p not in sys.path:
        sys.path.insert(0, _p)

P = 128


# ----------------------------------------------------------------------------
# Host-side preprocessing
# ----------------------------------------------------------------------------

def preprocess(edge_index, n_nodes, n_cores=8, slot_budget=64, max_group=8):
    """Build the ELL schedule and permutation.

    Returns dict with:
      order        [N]  orig node id by global degree rank (desc)
      new_of_orig  [N]  new id (core*NPC + local) per orig node
      npc          padded nodes per core (tiles*128)
      tpc          tiles per core
      groups       list of (tile0, ntiles, K) gather groups (same for all cores)
      S            total ELL columns
      ell          [n_cores, 128, S] int32 gather indices (new-id space, pad=ZROW)
      dinv         [n_cores, 128, tpc] f32
      zrow         dummy row index (= n_cores*npc)
    """
    src = np.asarray(edge_index[0], dtype=np.int64)
    dst = np.asarray(edge_index[1], dtype=np.int64)
    indeg = np.bincount(dst, minlength=n_nodes).astype(np.int64)
    deg = indeg + 1  # self loop
    dinv_all = (1.0 / np.sqrt(deg.astype(np.float64))).astype(np.float32)

    # global degree-descending order, dealt round-robin to cores
    order = np.argsort(-indeg, kind="stable")
    core_of_rank = np.arange(n_nodes) % n_cores
    local_of_rank = np.arange(n_nodes) // n_cores

    n_local = (n_nodes + n_cores - 1) // n_cores  # 12500
    tpc = (n_local + P - 1) // P                  # 98
    npc = tpc * P                                 # 12544
    npc1 = npc + 1  # each core's shard carries one trailing zero row
    zrow = npc      # core 0's zero row (pad slots gather zeros from here)

    new_of_orig = np.empty(n_nodes, dtype=np.int64)
    new_of_orig[order] = core_of_rank * npc1 + local_of_rank

    # per-tile K (max over cores): tile t covers global ranks [t*128*C, ...)
    indeg_sorted = indeg[order]
    K_tile = np.zeros(tpc, dtype=np.int64)
    for t in range(tpc):
        lo = t * P * n_cores
        K_tile[t] = int(indeg_sorted[lo]) if lo < n_nodes else 0
    K_tile = np.maximum(K_tile, 1)

    # greedy gather groups: consecutive tiles, ntiles*K <= slot_budget
    groups = []
    t = 0
    while t < tpc:
        K = int(K_tile[t])
        g = 1
        while (t + g < tpc and g < max_group
               and (g + 1) * K <= slot_budget):
            g += 1
        groups.append((t, g, K))
        t += g
    S = sum(g * K for (_, g, K) in groups)

    # ELL fill
    new_src = new_of_orig[src]
    new_dst = new_of_orig[dst]
    core_of_dst = new_dst // npc1
    local_dst = new_dst % npc1
    tile_of_dst = local_dst // P
    p_of_dst = local_dst % P

    # column base per tile
    colbase = np.zeros(tpc, dtype=np.int64)
    c = 0
    for (t0, g, K) in groups:
        for i in range(g):
            colbase[t0 + i] = c + i * K
        c += g * K
    assert c == S

    # rank of each edge among edges sharing its (new) dst
    order_e = np.argsort(new_dst, kind="stable")
    nd_sorted = new_dst[order_e]
    first = np.zeros(len(nd_sorted), dtype=np.int64)
    if len(nd_sorted):
        newgrp = np.ones(len(nd_sorted), dtype=bool)
        newgrp[1:] = nd_sorted[1:] != nd_sorted[:-1]
        grp_start = np.where(newgrp)[0]
        first = grp_start[np.cumsum(newgrp) - 1]
    rank_e = np.arange(len(nd_sorted)) - first

    ell = np.full((n_cores, P, S), zrow, dtype=np.int32)
    es = order_e
    cidx = core_of_dst[es]
    pidx = p_of_dst[es]
    col = colbase[tile_of_dst[es]] + rank_e
    ell[cidx, pidx, col] = new_src[es].astype(np.int32)

    dinv = np.zeros((n_cores, P, tpc), dtype=np.float32)
    # node new id c*npc + t*128 + p  -> orig id
    orig_of_new = np.full(n_cores * npc1, -1, dtype=np.int64)
    orig_of_new[new_of_orig] = np.arange(n_nodes)
    for cc in range(n_cores):
        ids = orig_of_new[cc * npc1:cc * npc1 + npc].reshape(tpc, P)
        valid = ids >= 0
        d = np.zeros((tpc, P), dtype=np.float32)
        d[valid] = dinv_all[ids[valid]]
        dinv[cc] = d.T  # [128, tpc]

    return dict(order=order, new_of_orig=new_of_orig, orig_of_new=orig_of_new,
                npc=npc, npc1=npc1, tpc=tpc, groups=groups, S=S, ell=ell,
                dinv=dinv, zrow=zrow, n_cores=n_cores)


def make_inputs(pre, x, W1, b1, W2, b2, W3, b3, Wf, bf):
    """Build per-core in_maps (list of dicts keyed by tensor name)."""
    n_cores, npc, tpc = pre["n_cores"], pre["npc"], pre["tpc"]
    F_in0 = x.shape[1]
    Fmax = W1.shape[1]  # 32
    orig_of_new = pre["orig_of_new"]

    # dinv broadcast to Fmax columns per tile: [128, tpc*Fmax]
    in_maps = []
    for c in range(n_cores):
        npc1 = pre["npc1"]
        xs = np.zeros((npc, F_in0), dtype=np.float32)
        ids = orig_of_new[c * npc1:c * npc1 + npc]
        valid = ids >= 0
        xs[valid] = np.asarray(x, dtype=np.float32)[ids[valid]]

        dinvb = np.repeat(pre["dinv"][c][:, :, None], Fmax, axis=2).reshape(P, tpc * Fmax)

        m = {
            "x": xs,
            "ell": pre["ell"][c],
            "dinv": np.ascontiguousarray(pre["dinv"][c]),
            "dinvb": np.ascontiguousarray(dinvb),
            "W1": np.asarray(W1, np.float32), "W2": np.asarray(W2, np.float32),
            "W3": np.asarray(W3, np.float32), "Wf": np.asarray(Wf, np.float32),
            "b1b": np.tile(np.asarray(b1, np.float32)[None, :], (P, 8)),
            "b2b": np.tile(np.asarray(b2, np.float32)[None, :], (P, 8)),
            "b3b": np.tile(np.asarray(b3, np.float32)[None, :], (P, 8)),
            "bfb": np.full((P, 1), np.float32(np.asarray(bf).reshape(-1)[0])),
            "ident": np.eye(P, dtype=np.float32),
        }
        in_maps.append(m)
    return in_maps


# ----------------------------------------------------------------------------
# Device kernel builder
# ----------------------------------------------------------------------------

def build(pre, feat_dims=(3, 32, 16, 8, 1), debug=False, gather_split=None):
    import concourse.bass as bass
    import concourse.bacc as bacc
    import concourse.tile as tile
    import concourse.mybir as mybir
    from concourse.bass import IndirectOffsetOnAxis

    f32 = mybir.dt.float32
    i32 = mybir.dt.int32

    n_cores, npc, tpc, S = pre["n_cores"], pre["npc"], pre["tpc"], pre["S"]
    npc1 = pre["npc1"]
    groups = pre["groups"]
    npad = n_cores * npc1
    F0, F1, F2, F3, F4 = feat_dims  # 3, 32, 16, 8, 1
    Fmax = F1

    nc = bacc.Bacc("TRN2", target_bir_lowering=False, debug=debug,
                   num_devices=n_cores)

    # --- parameters -------------------------------------------------------
    x_p = nc.declare_dram_parameter("x", [npc, F0], f32, isOutput=False)
    ell_p = nc.declare_dram_parameter("ell", [P, S], i32, isOutput=False)
    dinv_p = nc.declare_dram_parameter("dinv", [P, tpc], f32, isOutput=False)
    dinvb_p = nc.declare_dram_parameter("dinvb", [P, tpc * Fmax], f32, isOutput=False)
    W_p = {1: nc.declare_dram_parameter("W1", [F0, F1], f32, isOutput=False),
           2: nc.declare_dram_parameter("W2", [F1, F2], f32, isOutput=False),
           3: nc.declare_dram_parameter("W3", [F2, F3], f32, isOutput=False),
           4: nc.declare_dram_parameter("Wf", [F3, F4], f32, isOutput=False)}
    bb_p = {1: nc.declare_dram_parameter("b1b", [P, 8 * F1], f32, isOutput=False),
            2: nc.declare_dram_parameter("b2b", [P, 8 * F2], f32, isOutput=False),
            3: nc.declare_dram_parameter("b3b", [P, 8 * F3], f32, isOutput=False),
            4: nc.declare_dram_parameter("bfb", [P, 1], f32, isOutput=False)}
    ident_p = nc.declare_dram_parameter("ident", [P, P], f32, isOutput=False)
    out_p = nc.declare_dram_parameter("out", [npc, F4], f32, isOutput=True)

    rg = [list(range(n_cores))]
    layer_F = {1: (F0, F1), 2: (F1, F2), 3: (F2, F3)}

    with tile.TileContext(nc) as tc:
        with (
            tc.tile_pool(name="const", bufs=1) as constp,
            tc.tile_pool(name="acts", bufs=1) as actsp,
            tc.tile_pool(name="gath", bufs=10) as gathp,
            tc.tile_pool(name="work", bufs=2) as workp,
            tc.tile_pool(name="psumT", bufs=2, space="PSUM") as psumTp,
            tc.tile_pool(name="psumM", bufs=2, space="PSUM") as psumMp,
            tc.tile_pool(name="dram", bufs=1, space="DRAM") as dramp,
        ):
            # --- load constants to SBUF ----------------------------------
            ell_sb = constp.tile([P, S], i32)
            nc.sync.dma_start(ell_sb[:], ell_p[:])
            dinv_sb = constp.tile([P, tpc], f32)
            nc.sync.dma_start(dinv_sb[:], dinv_p[:])
            dinvb_sb = constp.tile([P, tpc * Fmax], f32)
            nc.sync.dma_start(dinvb_sb[:], dinvb_p[:])
            ident_sb = constp.tile([P, P], f32)
            nc.sync.dma_start(ident_sb[:], ident_p[:])
            W_sb = {}
            bb_sb = {}
            for l in (1, 2, 3, 4):
                fin, fout = (layer_F[l] if l != 4 else (F3, F4))
                W_sb[l] = constp.tile([fin, fout], f32, name=f"W{l}_sb")
                nc.sync.dma_start(W_sb[l][:], W_p[l][:])
                bw = 8 * fout if l != 4 else 1
                bb_sb[l] = constp.tile([P, bw], f32, name=f"bb{l}_sb")
                nc.sync.dma_start(bb_sb[l][:], bb_p[l][:])

            zero_sb = constp.tile([1, Fmax], f32)
            nc.vector.memset(zero_sb[:], 0.0)

            # --- activations ---------------------------------------------
            a_cur = actsp.tile([P, tpc * F0], f32, name="a1")
            # x [npc, F0] rows t*128+p -> a_cur[p, t*F0 + f]
            nc.sync.dma_start(
                a_cur[:],
                x_p[:].rearrange("(t p) f -> p t f", p=P),
            )

            g_tab = {}
            for l in (1, 2, 3):
                fout = layer_F[l][1]
                g_tab[l] = dramp.tile([npad, fout], f32, name=f"g_tab{l}",
                                      addr_space="Shared")

            # ================= layers =====================================
            for l in (1, 2, 3):
                fin, fout = layer_F[l]
                g_in = dramp.tile([npc1, fout], f32, name=f"g_in{l}")
                # trailing zero row of this core's shard (gather pad target)
                nc.sync.dma_start(g_in[npc:npc1, :], zero_sb[:1, :fout])
                g_own = actsp.tile([P, tpc * fout], f32, name=f"g_own{l}")

                # ---- stage A: g_own = dinv * (a_cur @ W) ----------------
                for t in range(tpc):
                    a_sc = workp.tile([P, fin], f32, name=f"a_sc{l}", tag="a_sc")
                    nc.vector.tensor_scalar_mul(
                        a_sc[:], a_cur[:, t * fin:(t + 1) * fin],
                        dinv_sb[:, t:t + 1])
                    aT_ps = psumTp.tile([fin, P], f32, name=f"aT{l}", tag="aT")
                    nc.tensor.transpose(aT_ps[:], a_sc[:], ident_sb[:])
                    aT_sb = workp.tile([fin, P], f32, name=f"aTs{l}", tag="aTs")
                    nc.scalar.copy(aT_sb[:], aT_ps[:])
                    g_ps = psumMp.tile([P, fout], f32, name=f"g_ps{l}", tag="g_ps")
                    nc.tensor.matmul(g_ps[:], aT_sb[:], W_sb[l][:],
                                     start=True, stop=True)
                    nc.scalar.copy(g_own[:, t * fout:(t + 1) * fout], g_ps[:])

                # DMA g_own -> g_in in batches of 8 tiles
                for t0 in range(0, tpc, 8):
                    nt = min(8, tpc - t0)
                    nc.sync.dma_start(
                        g_in[t0 * P:(t0 + nt) * P, :].rearrange(
                            "(t p) f -> p t f", p=P),
                        g_own[:, t0 * fout:(t0 + nt) * fout],
                    )

                # ---- stage B: allgather ---------------------------------
                nc.gpsimd.collective_compute(
                    "AllGather", mybir.AluOpType.bypass,
                    replica_groups=rg,
                    ins=[g_in[:]],
                    outs=[g_tab[l][:]],
                )

                # ---- stage C: gather + accumulate -----------------------
                a_next = actsp.tile([P, tpc * fout], f32, name=f"a{l + 1}")
                for (t0, gsz, K) in groups:
                    cols = gsz * K
                    c0 = None  # column base of this group in ell
                    # column base: sum of previous groups
                    gat = gathp.tile([P, 64 * Fmax], f32, name=f"gat{l}",
                                     tag="gat")
                    # compute ell col base
                    cb = 0
                    for (tt0, gg, KK) in groups:
                        if tt0 == t0:
                            break
                        cb += gg * KK
                    gs = gather_split or cols
                    for c_off in range(0, cols, gs):
                        cw = min(gs, cols - c_off)
                        nc.gpsimd.indirect_dma_start(
                            out=gat[:, c_off * fout:(c_off + cw) * fout],
                            out_offset=None,
                            in_=g_tab[l][:],
                            in_offset=IndirectOffsetOnAxis(
                                ap=ell_sb[:, cb + c_off:cb + c_off + cw], axis=0),
                        )
                    # tree-accumulate within each tile: view [P, gsz, K, F]
                    def gslice(k_lo, k_cnt):
                        # AP [P, gsz, k_cnt, fout] at slot offset k_lo
                        return gat[:, 0:gsz * K * fout].rearrange(
                            "p (g k f) -> p g k f", g=gsz, k=K
                        )[:, :, k_lo:k_lo + k_cnt, :]

                    k = K
                    while k > 1:
                        h = (k + 1) // 2
                        cnt = k - h
                        nc.vector.tensor_tensor(
                            out=gslice(0, cnt), in0=gslice(0, cnt),
                            in1=gslice(h, cnt), op=mybir.AluOpType.add)
                        k = h
                    # finish: a_next_slice = relu(dinv*(acc + g_own) + b)
                    dst = a_next[:, t0 * fout:(t0 + gsz) * fout]
                    nc.vector.tensor_tensor(
                        out=dst, in0=gslice(0, 1).squeeze(),
                        in1=g_own[:, t0 * fout:(t0 + gsz) * fout],
                        op=mybir.AluOpType.add)
                    dv = dinvb_sb[:, 0:tpc * Fmax].rearrange(
                        "p (t f) -> p t f", f=Fmax)[:, t0:t0 + gsz, 0:fout]
                    nc.vector.tensor_tensor(out=dst, in0=dst, in1=dv,
                                            op=mybir.AluOpType.mult)
                    nc.vector.tensor_tensor(out=dst, in0=dst,
                                            in1=bb_sb[l][:, 0:gsz * fout],
                                            op=mybir.AluOpType.add)
                    nc.vector.tensor_scalar_max(dst, dst, 0.0)
                a_cur = a_next

            # ================= final linear ===============================
            y_sb = actsp.tile([P, tpc], f32)
            for t in range(tpc):
                aT_ps = psumTp.tile([F3, P], f32, name="aTf", tag="aT")
                nc.tensor.transpose(aT_ps[:], a_cur[:, t * F3:(t + 1) * F3],
                                    ident_sb[:])
                aT_sb = workp.tile([F3, P], f32, name="aTfs", tag="aTs")
                nc.scalar.copy(aT_sb[:], aT_ps[:])
                y_ps = psumMp.tile([P, F4], f32, name="y_ps", tag="g_ps")
                nc.tensor.matmul(y_ps[:], aT_sb[:], W_sb[4][:],
                                 start=True, stop=True)
                nc.vector.tensor_scalar_add(y_sb[:, t:t + 1], y_ps[:],
                                            bb_sb[4][:, 0:1])
            nc.sync.dma_start(
                out_p[:].rearrange("(t p) f -> p t f", p=P),
                y_sb[:],
            )

    nc.compile()
    return nc


# ----------------------------------------------------------------------------
# Full pipeline
# ----------------------------------------------------------------------------

def _ensure_ntff_hook():
    import sys, types
    try:
        from antenv.axon_hooks import get_axon_ntff_profile_hook  # noqa
        return
    except ImportError:
        pass
    mod = types.ModuleType("antenv.axon_hooks")
    _h = {"h": None}
    mod.set_axon_ntff_profile_hook = lambda h: _h.__setitem__("h", h)
    mod.get_axon_ntff_profile_hook = lambda: _h["h"]
    import antenv
    sys.modules["antenv.axon_hooks"] = mod
    antenv.axon_hooks = mod
    try:
        from trn_agent_boot.trn_boot import _ntff_profile_via_ctypes
        mod.set_axon_ntff_profile_hook(
            _ntff_profile_via_ctypes("/opt/axon/libaxon_pjrt.so"))
    except Exception:
        pass


def run(x, edge_index, W1, b1, W2, b2, W3, b3, Wf, bf, *, trace=False,
        n_cores=8, use_sim=False, tmpdir=None, gather_split=None):
    import concourse.bass_utils as bass_utils
    from concourse.bass_utils import run_bass_kernel_spmd
    if trace:
        _ensure_ntff_hook()
        bass_utils.upload_artifacts = lambda d: d  # no bucket in container

    n_nodes = x.shape[0]
    pre = preprocess(edge_index, n_nodes, n_cores=n_cores)
    in_maps = make_inputs(pre, x, W1, b1, W2, b2, W3, b3, Wf, bf)
    nc = build(pre, feat_dims=(x.shape[1], W1.shape[1], W2.shape[1],
                               W3.shape[1], Wf.shape[1]),
               gather_split=gather_split)

    if use_sim:
        import concourse.bass_interp as bass_interp
        sim = bass_interp.MultiCoreSim(nc, n_cores)
        for c in range(n_cores):
            for k, v in in_maps[c].items():
                sim.cores[c].tensor(k)[:] = v
        sim.simulate(check_with_hw=False)
        results = [{"out": np.array(sim.cores[c].mem_tensor("out"))}
                   for c in range(n_cores)]
        res = None
    else:
        res = run_bass_kernel_spmd(nc, in_maps, list(range(n_cores)),
                                   trace=trace, tmpdir=tmpdir)
        results = res.results

    npc1 = pre["npc1"]
    y_new = np.concatenate(
        [np.concatenate([results[c]["out"],
                         np.zeros((1, results[c]["out"].shape[1]),
                                  results[c]["out"].dtype)], axis=0)
         for c in range(n_cores)], axis=0)
    assert y_new.shape[0] == n_cores * npc1
    y = y_new[pre["new_of_orig"]]
    return y, res, pre



def kernel(x, edge_index, W1, b1, W2, b2, W3, b3, Wf, bf):
    """Entry point for the grading harness: full inputs in, full output out.

    gather_split=1 is required: the HW indirect DMA consumes one index per
    partition per instruction (multi-column offset APs are wrong on HW).
    """
    y, _, _ = run(x, edge_index, W1, b1, W2, b2, W3, b3, Wf, bf,
                  trace=False, gather_split=1)
    return np.asarray(y, dtype=np.float32)

